# revision 15
# baseline (speedup 1.0000x reference)
"""TRN2 Bass kernel for CrossAttention (B=16, L=1024, H=A=1024, fp32).

Strategy (8 NeuronCores, data-parallel over batch, 2 batch elements/core),
with algebraic fusion to avoid weight transposes and one projection:

  scores = (meme Wq^T + bq)(text Wk^T + bk)^T ; softmax over k ; @ (emoji Wv^T + bv)

  1. bk shifts every softmax row by a constant -> drops out exactly.
  2. Mt[h2,h] = sum_a Wq[a,h2] Wk[a,h] is computed ONCE from both weights in
     natural layout (contraction over a = partition dim).  Then per batch:
        G[h,q]  = sum_h2 Mt[h2,h] meme^T[h2,q] + c[h]   (c = Wk^T bq)
        S^T[k,q] = sum_h text^T[h,k] G[h,q]             == Q K0^T transposed
  3. softmax skips max-subtraction (logits bounded ~83; exp fits fp32/bf16),
     E^T = exp(S^T) in bf16 straight out of PSUM on the Scalar engine.
  4. V-projection is fused into the output:  O = (E/s) emoji Wv^T + bv:
        T^T[h,q] = sum_k emoji[k,h] E^T[k,q]   (emoji natural, bf16 - no transpose)
        O[q,a]   = sum_h T^T[h,q] WvT[h,a]     (WvT transposed once, bf16)
        row sums s[q] via N=1 matmuls vs a ones vector; final scale+bias
        on the PSUM->SBUF copy (ACT scale=1/s, GPSIMD +bv).

  Precision plan: the logit path (Mt, memeT/textT, G) is fp16 (2.8e-4 rms
  per quantization step -> ~0.6% output error; fp32 PSUM accumulate), the
  output path (E, emoji, T, WvT) is bf16 for exp range.  fp16/bf16 matmuls
  stream at 1 cyc/row with 2-byte LDWEIGHTS that hide fully; f32r is kept
  only where data arrives raw from DRAM (transposes, Mt inputs).

  Engine balance per batch: PE ~124us; ACT drains feature transposes + exp
  + O-scale (~47us); DVE drains Mt/G/T + EM casts (~35us); GPSIMD adds bv
  (~17us).  PSUM: 2 banks transposes / 4 banks matmul groups / 2 small.
"""

import sys

sys.path.insert(0, "/opt/trn_rl_repo")

import contextlib
import numpy as np
import concourse.bacc as bacc
import concourse.bass as bass
import concourse.mybir as mybir
from concourse.tile import TileContext
from concourse.bass_utils import run_bass_kernel_spmd
from concourse.masks import make_identity

F32 = mybir.dt.float32
F32R = mybir.dt.float32r
F16 = mybir.dt.float16
BF16 = mybir.dt.bfloat16
EXP = mybir.ActivationFunctionType.Exp
COPY = mybir.ActivationFunctionType.Copy

P = 128
B, L, H, A = 16, 1024, 1024, 1024
NCORES = 8
NB = B // NCORES  # batch elements per core
NH = H // P       # 8 chunks


def _build_program(repeat=1):
    nc = bacc.Bacc("TRN2", target_bir_lowering=False, debug=False, num_devices=NCORES)

    xm = nc.declare_dram_parameter("xm", [NB, L, H], F32R, isOutput=False)
    xt_ = nc.declare_dram_parameter("xt", [NB, L, H], F32R, isOutput=False)
    xe = nc.declare_dram_parameter("xe", [NB, L, H], F32, isOutput=False)
    # per-core weight slices: core i gets Wq/Wk columns [i*128, (i+1)*128)
    wqs = nc.declare_dram_parameter("wqs", [A, P], F32R, isOutput=False)
    wks = nc.declare_dram_parameter("wks", [A, P], F32R, isOutput=False)
    wk = nc.declare_dram_parameter("wk", [A, H], F32R, isOutput=False)
    wv = nc.declare_dram_parameter("wv", [A, H], F32R, isOutput=False)
    bq = nc.declare_dram_parameter("bq", [A], F32R, isOutput=False)
    bv = nc.declare_dram_parameter("bv", [A], F32, isOutput=False)
    o = nc.declare_dram_parameter("o", [NB, L, A], F32, isOutput=True)
    # DRAM scratch for the Mt AllGather: each core contributes its 128-row
    # slice of Mt (cols 0:1024) plus its 128-entry slice of c (col 1024).
    mt_sl = nc.dram_tensor("mt_sl", [P, H + 2], F16)
    mt_all = nc.dram_tensor("mt_all", [NCORES, P, H + 2], F16, addr_space="Shared")

    with TileContext(nc) as tc:
        with contextlib.ExitStack() as stack:
            ep = stack.enter_context
            sgl = ep(tc.tile_pool(name="sgl", bufs=1))
            mtp = ep(tc.tile_pool(name="mt", bufs=8))
            wvtp = ep(tc.tile_pool(name="wvt", bufs=8))
            smp = ep(tc.tile_pool(name="sm", bufs=4))
            xmp = ep(tc.tile_pool(name="xm", bufs=8))
            xtp = ep(tc.tile_pool(name="xt", bufs=8))
            gp = ep(tc.tile_pool(name="g", bufs=8))
            emp = ep(tc.tile_pool(name="em", bufs=8))
            etp = ep(tc.tile_pool(name="et", bufs=16))
            ttp = ep(tc.tile_pool(name="tt", bufs=8))
            estp = ep(tc.tile_pool(name="est", bufs=2))
            opp = ep(tc.tile_pool(name="op", bufs=4))
            blk = ep(tc.tile_pool(name="blk", bufs=6))
            pstp = ep(tc.tile_pool(name="pst", bufs=2, space="PSUM"))
            psp = ep(tc.tile_pool(name="mm", bufs=4, space="PSUM"))
            ps2 = ep(tc.tile_pool(name="ps2", bufs=2, space="PSUM"))
            rep_ctx = tc.For_i(0, repeat, 1) if repeat > 1 else contextlib.nullcontext()
            with rep_ctx:
                ident_f = sgl.tile([P, P], F32, tag="ident_f")
                make_identity(nc, ident_f)
                ident = sgl.tile([P, P], F32R, tag="ident")
                nc.vector.tensor_copy(ident, ident_f)
                bvb = sgl.tile([P, A], F32, tag="bvb")
                nc.sync.dma_start(out=bvb, in_=bv.ap().partition_broadcast(P))
                bqc = sgl.tile([P, NH + 1], F32R, tag="bqc")
                zrow = sgl.tile([P, 1], F32, tag="zrow")
                nc.vector.memset(zrow, 0.0)
                nc.vector.tensor_copy(bqc[:, NH : NH + 1], zrow)
                nc.sync.dma_start(
                    out=bqc[:, 0:NH], in_=bq.ap().rearrange("(c p) -> p c", p=P)
                )
                ones_bf = sgl.tile([P, 1], BF16, tag="ones_bf")
                nc.vector.memset(ones_bf, 1.0)
                cT = sgl.tile([P, NH], F32, tag="cT")

                def feat_transpose(x_dram, b, pool, tag):
                    """[L, H] natural f32r -> 8 tiles X^T[h_chunk] of [128, L] fp16."""
                    tiles = []
                    for hc in range(NH):
                        t = pool.tile([P, L], F16, tag=tag)
                        for g in range(2):
                            pst = pstp.tile([P, 512], F32R, tag="tp")
                            bt = blk.tile([P, 4, P], F32R, tag="blk")
                            nc.sync.dma_start(
                                out=bt,
                                in_=x_dram.ap()[
                                    b, g * 512 : (g + 1) * 512, hc * P : (hc + 1) * P
                                ].rearrange("(c p) h -> p c h", p=P),
                            )
                            for j in range(4):
                                nc.tensor.transpose(
                                    pst[:, j * P : (j + 1) * P], bt[:, j, :], ident
                                )
                            nc.scalar.activation(
                                t[:, g * 512 : (g + 1) * 512], pst, COPY
                            )
                        tiles.append(t)
                    return tiles

                Mt = [mtp.tile([P, H], F16, tag="mt", name=f"mt{i}") for i in range(NH)]
                WvT = [
                    wvtp.tile([P, A], BF16, tag="wvt", name=f"wvt{i}")
                    for i in range(NH)
                ]

                # ---- one-time, sharded: this core computes Mt rows
                # [id*128,(id+1)*128) = Wq[:, slice]^T Wk (16 matmuls) and the
                # matching c slice = Wk[:, slice]^T bq, packs both into a
                # [128, 1026] fp16 DRAM tile, AllGathers across the 8 cores,
                # then reads the full Mt (8 tiles) + cT back.  The gather
                # latency hides under the batch-0 feature transposes.
                with (
                    tc.tile_pool(name="wq", bufs=8) as wqp,
                    tc.tile_pool(name="wk", bufs=10) as wkp,
                ):
                    wqst = []
                    wkst = []
                    for ci in range(NH):
                        tq = wqp.tile([P, P], F32R, tag="wqs", name=f"wqs{ci}")
                        nc.sync.dma_start(
                            out=tq, in_=wqs.ap()[ci * P : (ci + 1) * P, :]
                        )
                        wqst.append(tq)
                        tk = wqp.tile([P, P], F32R, tag="wks", name=f"wks{ci}")
                        nc.sync.dma_start(
                            out=tk, in_=wks.ap()[ci * P : (ci + 1) * P, :]
                        )
                        wkst.append(tk)
                    stg = sgl.tile([P, H + 2], F16, tag="mtstg")
                    # c slice: [128, 1] = sum_a Wk[a, slice] bq[a]
                    psc = ps2.tile([P, 2], F32, tag="sum")
                    for ac in range(NH):
                        nc.tensor.matmul(
                            psc,
                            lhsT=wkst[ac],
                            rhs=bqc[:, ac : ac + 2],
                            start=(ac == 0),
                            stop=(ac == NH - 1),
                        )
                    nc.vector.tensor_copy(stg[:, H : H + 1], psc[:, 0:1])
                    nc.vector.tensor_copy(stg[:, H + 1 : H + 2], psc[:, 1:2])
                    for g in range(2):
                        wkh = []
                        for ci in range(NH):
                            tk = wkp.tile([P, 512], F32R, tag="wk")
                            nc.sync.dma_start(
                                out=tk,
                                in_=wk.ap()[
                                    ci * P : (ci + 1) * P, g * 512 : (g + 1) * 512
                                ],
                            )
                            wkh.append(tk)
                        pst = psp.tile([P, 512], F32, tag="mm")
                        for ac in range(NH):
                            nc.tensor.matmul(
                                pst,
                                lhsT=wqst[ac],
                                rhs=wkh[ac],
                                start=(ac == 0),
                                stop=(ac == NH - 1),
                            )
                        nc.vector.tensor_copy(stg[:, g * 512 : (g + 1) * 512], pst)
                    nc.sync.dma_start(out=mt_sl[:], in_=stg)
                    nc.gpsimd.collective_compute(
                        "AllGather",
                        mybir.AluOpType.bypass,
                        replica_groups=[list(range(NCORES))],
                        ins=[mt_sl[:].opt()],
                        outs=[mt_all[:].opt()],
                    )
                    for h2 in range(NH):
                        nc.sync.dma_start(
                            out=Mt[h2], in_=mt_all.ap()[h2, :, 0:H]
                        )
                    cT16 = sgl.tile([P, NH], F16, tag="cT16")
                    nc.sync.dma_start(
                        out=cT16,
                        in_=mt_all.ap()[:, :, H : H + 1].rearrange("g p x -> p (g x)"),
                    )
                    nc.vector.tensor_copy(cT, cT16)

                # ---- batch 0 feature transposes (overlap the Mt gather) ----
                memeT = feat_transpose(xm, 0, xmp, "xmt")

                def compute_G(memeT):
                    G = []
                    for ht in range(NH):
                        gt = gp.tile([P, L], F16, tag="g")
                        for qb in range(2):
                            pst = psp.tile([P, 512], F32, tag="mm")
                            for h2 in range(NH):
                                nc.tensor.matmul(
                                    pst,
                                    lhsT=Mt[h2][:, ht * P : (ht + 1) * P],
                                    rhs=memeT[h2][:, qb * 512 : (qb + 1) * 512],
                                    start=(h2 == 0),
                                    stop=(h2 == NH - 1),
                                )
                            nc.vector.tensor_scalar_add(
                                gt[:, qb * 512 : (qb + 1) * 512],
                                pst,
                                cT[:, ht : ht + 1],
                            )
                        G.append(gt)
                    return G

                def load_emoji(b):
                    EM = []
                    for kc in range(NH):
                        emt = emp.tile([P, H], BF16, tag="em")
                        for g in range(2):
                            est = estp.tile([P, 512], F32, tag="estg")
                            nc.sync.dma_start(
                                out=est,
                                in_=xe.ap()[
                                    b, kc * P : (kc + 1) * P, g * 512 : (g + 1) * 512
                                ],
                            )
                            nc.vector.tensor_copy(
                                emt[:, g * 512 : (g + 1) * 512], est
                            )
                        EM.append(emt)
                    return EM

                def attention_qb(b, qb, textT, G, EM):
                    if True:
                        # S^T[k_tile, qb] -> exp -> E^T bf16
                        ets = []
                        for kt in range(NH):
                            pst = psp.tile([P, 512], F32, tag="mm")
                            for hc in range(NH):
                                nc.tensor.matmul(
                                    pst,
                                    lhsT=textT[hc][:, kt * P : (kt + 1) * P],
                                    rhs=G[hc][:, qb * 512 : (qb + 1) * 512],
                                    start=(hc == 0),
                                    stop=(hc == NH - 1),
                                )
                            e_t = etp.tile([P, 512], BF16, tag="et")
                            nc.scalar.activation(e_t, pst, EXP)
                            ets.append(e_t)

                        # T^T[h_tile, qb] = sum_k emoji[k, h] E^T[k, qb]
                        Tt = []
                        for ht in range(NH):
                            pst = psp.tile([P, 512], F32, tag="mm")
                            for kc in range(NH):
                                nc.tensor.matmul(
                                    pst,
                                    lhsT=EM[kc][:, ht * P : (ht + 1) * P],
                                    rhs=ets[kc],
                                    start=(kc == 0),
                                    stop=(kc == NH - 1),
                                )
                            t_t = ttp.tile([P, 512], BF16, tag="tt")
                            nc.vector.tensor_copy(t_t, pst)
                            Tt.append(t_t)

                        # O[q_tile, :] = (sum_h T^T[h,q] WvT[h,a]) / s[q] + bv
                        for qt in range(4):
                            qs = qt * P
                            ps0 = psp.tile([P, 512], F32, tag="mm")
                            ps1 = psp.tile([P, 512], F32, tag="mm")
                            pss = ps2.tile([P, 1], F32, tag="sum")
                            for kc in range(NH):
                                nc.tensor.matmul(
                                    pss,
                                    lhsT=ets[kc][:, qs : qs + P],
                                    rhs=ones_bf,
                                    start=(kc == 0),
                                    stop=(kc == NH - 1),
                                )
                            for hc in range(NH):
                                st, sp = (hc == 0), (hc == NH - 1)
                                nc.tensor.matmul(
                                    ps0,
                                    lhsT=Tt[hc][:, qs : qs + P],
                                    rhs=WvT[hc][:, 0:512],
                                    start=st,
                                    stop=sp,
                                )
                                nc.tensor.matmul(
                                    ps1,
                                    lhsT=Tt[hc][:, qs : qs + P],
                                    rhs=WvT[hc][:, 512:1024],
                                    start=st,
                                    stop=sp,
                                )
                            rec = smp.tile([P, 1], F32, tag="rec")
                            nc.vector.reciprocal(rec, pss)
                            q0 = qb * 512 + qs
                            for g, psg in ((0, ps0), (1, ps1)):
                                o_t = opp.tile([P, 512], F32, tag="op")
                                nc.scalar.activation(o_t, psg, COPY, scale=rec)
                                nc.vector.tensor_add(
                                    o_t, o_t, bvb[:, g * 512 : (g + 1) * 512]
                                )
                                nc.scalar.dma_start(
                                    out=o.ap()[
                                        b, q0 : q0 + P, g * 512 : (g + 1) * 512
                                    ],
                                    in_=o_t,
                                )

                # ---- batch 0 ----
                G0 = compute_G(memeT)
                textT = feat_transpose(xt_, 0, xtp, "xtt")
                # WvT via PE transpose of Wv natural blocks (needed by O stage)
                for hc in range(NH):
                    for g in range(2):
                        pst = pstp.tile([P, 512], F32R, tag="tp")
                        bt = blk.tile([P, 4, P], F32R, tag="blk")
                        nc.sync.dma_start(
                            out=bt,
                            in_=wv.ap()[
                                g * 512 : (g + 1) * 512, hc * P : (hc + 1) * P
                            ].rearrange("(c p) h -> p c h", p=P),
                        )
                        for j in range(4):
                            nc.tensor.transpose(
                                pst[:, j * P : (j + 1) * P], bt[:, j, :], ident
                            )
                        nc.vector.tensor_copy(
                            WvT[hc][:, g * 512 : (g + 1) * 512], pst
                        )
                EM = load_emoji(0)
                attention_qb(0, 0, textT, G0, EM)
                # prefetch batch-1 meme transposes between batch-0 qb passes
                memeT1 = feat_transpose(xm, 1, xmp, "xmt")
                attention_qb(0, 1, textT, G0, EM)

                # ---- batch 1 ----
                G1 = compute_G(memeT1)
                textT1 = feat_transpose(xt_, 1, xtp, "xtt")
                EM1 = load_emoji(1)
                attention_qb(1, 0, textT1, G1, EM1)
                attention_qb(1, 1, textT1, G1, EM1)

    nc.compile()
    return nc


_NC = {}


def _get_nc(repeat=1):
    if repeat not in _NC:
        _NC[repeat] = _build_program(repeat)
    return _NC[repeat]


def _run(inputs, trace=False, repeat=1):
    nc = _get_nc(repeat)
    c = np.ascontiguousarray

    def f32c(x):
        return c(np.asarray(x, dtype=np.float32))

    meme = f32c(inputs["meme_features"])
    text = f32c(inputs["text_features"])
    emoji = f32c(inputs["emoji_features"])
    wq_f = f32c(inputs["Wq"])
    wk_f = f32c(inputs["Wk"])
    full = {
        "wk": wk_f,
        "wv": f32c(inputs["Wv"]),
        "bq": f32c(inputs["bq"]),
        "bv": f32c(inputs["bv"]),
    }
    in_maps = []
    for i in range(NCORES):
        s = slice(i * NB, (i + 1) * NB)
        in_maps.append(
            {
                "xm": c(meme[s]),
                "xt": c(text[s]),
                "xe": c(emoji[s]),
                "wqs": c(wq_f[:, i * P : (i + 1) * P]),
                "wks": c(wk_f[:, i * P : (i + 1) * P]),
                **full,
            }
        )
    res = run_bass_kernel_spmd(nc, in_maps, list(range(NCORES)), trace=trace)
    out = np.concatenate([res.results[i]["o"] for i in range(NCORES)], axis=0)
    return out, res


def kernel(**inputs):
    out, _ = _run(inputs, trace=False)
    return out


if __name__ == "__main__":
    rng = np.random.default_rng(0)
    s = 1.0 / np.sqrt(H)
    inputs = {
        "meme_features": rng.standard_normal((B, L, H), dtype=np.float32),
        "text_features": rng.standard_normal((B, L, H), dtype=np.float32),
        "emoji_features": rng.standard_normal((B, L, H), dtype=np.float32),
        "Wq": rng.uniform(-s, s, (A, H)).astype(np.float32),
        "bq": rng.uniform(-s, s, A).astype(np.float32),
        "Wk": rng.uniform(-s, s, (A, H)).astype(np.float32),
        "bk": rng.uniform(-s, s, A).astype(np.float32),
        "Wv": rng.uniform(-s, s, (A, H)).astype(np.float32),
        "bv": rng.uniform(-s, s, A).astype(np.float32),
    }
    out = kernel(**inputs)
    q = np.einsum("blh,ah->bla", inputs["meme_features"], inputs["Wq"]) + inputs["bq"]
    k = np.einsum("blh,ah->bla", inputs["text_features"], inputs["Wk"]) + inputs["bk"]
    v = np.einsum("blh,ah->bla", inputs["emoji_features"], inputs["Wv"]) + inputs["bv"]
    sc = np.einsum("bqa,bka->bqk", q, k)
    sc -= sc.max(-1, keepdims=True)
    w = np.exp(sc)
    w /= w.sum(-1, keepdims=True)
    ref = np.einsum("bqk,bka->bqa", w, v)
    err = np.linalg.norm(out - ref) / np.linalg.norm(ref)
    print(f"smoke rel err: {err:.3e}")


# revision 20
# speedup vs baseline: 1.3392x; 1.3392x over previous
"""TRN2 Bass kernel for CrossAttention (B=16, L=1024, H=A=1024, fp32).

Strategy (8 NeuronCores, data-parallel over batch, 2 batch elements/core),
with algebraic fusion to avoid weight transposes and one projection:

  scores = (meme Wq^T + bq)(text Wk^T + bk)^T ; softmax over k ; @ (emoji Wv^T + bv)

  1. bk shifts every softmax row by a constant -> drops out exactly.
  2. Mt[h2,h] = sum_a Wq[a,h2] Wk[a,h] is computed ONCE from both weights in
     natural layout (contraction over a = partition dim).  Then per batch:
        G[h,q]  = sum_h2 Mt[h2,h] meme^T[h2,q] + c[h]   (c = Wk^T bq)
        S^T[k,q] = sum_h text^T[h,k] G[h,q]             == Q K0^T transposed
  3. softmax skips max-subtraction (logits bounded ~83; exp fits fp32/bf16),
     E^T = exp(S^T) in bf16 straight out of PSUM on the Scalar engine.
  4. V-projection is fused into the output:  O = (E/s) emoji Wv^T + bv:
        T^T[h,q] = sum_k emoji[k,h] E^T[k,q]   (emoji natural, bf16 - no transpose)
        O[q,a]   = sum_h T^T[h,q] WvT[h,a]     (WvT transposed once, bf16)
        row sums s[q] via N=1 matmuls vs a ones vector; final scale+bias
        on the PSUM->SBUF copy (ACT scale=1/s, GPSIMD +bv).

  Precision plan: the logit path (Mt, memeT/textT, G) is fp16 (2.8e-4 rms
  per quantization step -> ~0.6% output error; fp32 PSUM accumulate), the
  output path (E, emoji, T, WvT) is bf16 for exp range.  fp16/bf16 matmuls
  stream at 1 cyc/row with 2-byte LDWEIGHTS that hide fully; f32r is kept
  only where data arrives raw from DRAM (transposes, Mt inputs).

  Engine balance per batch: PE ~124us; ACT drains feature transposes + exp
  + O-scale (~47us); DVE drains Mt/G/T + EM casts (~35us); GPSIMD adds bv
  (~17us).  PSUM: 2 banks transposes / 4 banks matmul groups / 2 small.
"""

import sys

sys.path.insert(0, "/opt/trn_rl_repo")

import contextlib
import numpy as np
import concourse.bacc as bacc
import concourse.bass as bass
import concourse.mybir as mybir
from concourse.tile import TileContext
from concourse.bass_utils import run_bass_kernel_spmd
from concourse.masks import make_identity

F32 = mybir.dt.float32
F32R = mybir.dt.float32r
F16 = mybir.dt.float16
BF16 = mybir.dt.bfloat16
EXP = mybir.ActivationFunctionType.Exp
COPY = mybir.ActivationFunctionType.Copy

P = 128
B, L, H, A = 16, 1024, 1024, 1024
NCORES = 8
NB = B // NCORES  # batch elements per core
NH = H // P       # 8 chunks


def _build_program(repeat=1):
    nc = bacc.Bacc("TRN2", target_bir_lowering=False, debug=False, num_devices=NCORES)

    xm = nc.declare_dram_parameter("xm", [NB, L, H], F32R, isOutput=False)
    xt_ = nc.declare_dram_parameter("xt", [NB, L, H], F32R, isOutput=False)
    xe = nc.declare_dram_parameter("xe", [NB, L, H], F32, isOutput=False)
    wq = nc.declare_dram_parameter("wq", [A, H], F32R, isOutput=False)
    wk = nc.declare_dram_parameter("wk", [A, H], F32R, isOutput=False)
    wv = nc.declare_dram_parameter("wv", [A, H], F32R, isOutput=False)
    bq = nc.declare_dram_parameter("bq", [A], F32R, isOutput=False)
    bv = nc.declare_dram_parameter("bv", [A], F32, isOutput=False)
    o = nc.declare_dram_parameter("o", [NB, L, A], F32, isOutput=True)

    with TileContext(nc) as tc:
        with contextlib.ExitStack() as stack:
            ep = stack.enter_context
            sgl = ep(tc.tile_pool(name="sgl", bufs=1))
            mtp = ep(tc.tile_pool(name="mt", bufs=8))
            wvtp = ep(tc.tile_pool(name="wvt", bufs=8))
            smp = ep(tc.tile_pool(name="sm", bufs=4))
            xmp = ep(tc.tile_pool(name="xm", bufs=8))
            xtp = ep(tc.tile_pool(name="xt", bufs=8))
            gp = ep(tc.tile_pool(name="g", bufs=8))
            emp = ep(tc.tile_pool(name="em", bufs=8))
            etp = ep(tc.tile_pool(name="et", bufs=16))
            ttp = ep(tc.tile_pool(name="tt", bufs=8))
            estp = ep(tc.tile_pool(name="est", bufs=2))
            opp = ep(tc.tile_pool(name="op", bufs=4))
            blk = ep(tc.tile_pool(name="blk", bufs=4))
            pstp = ep(tc.tile_pool(name="pst", bufs=2, space="PSUM"))
            psp = ep(tc.tile_pool(name="mm", bufs=4, space="PSUM"))
            ps2 = ep(tc.tile_pool(name="ps2", bufs=2, space="PSUM"))
            rep_ctx = tc.For_i(0, repeat, 1) if repeat > 1 else contextlib.nullcontext()
            with rep_ctx:
                ident_f = sgl.tile([P, P], F32, tag="ident_f")
                make_identity(nc, ident_f)
                ident = sgl.tile([P, P], F32R, tag="ident")
                nc.vector.tensor_copy(ident, ident_f)
                bvb = sgl.tile([P, A], F32, tag="bvb")
                nc.sync.dma_start(out=bvb, in_=bv.ap().partition_broadcast(P))
                bqc = sgl.tile([P, NH + 1], F32R, tag="bqc")
                zrow = sgl.tile([P, 1], F32, tag="zrow")
                nc.vector.memset(zrow, 0.0)
                nc.vector.tensor_copy(bqc[:, NH : NH + 1], zrow)
                nc.sync.dma_start(
                    out=bqc[:, 0:NH], in_=bq.ap().rearrange("(c p) -> p c", p=P)
                )
                ones_bf = sgl.tile([P, 1], BF16, tag="ones_bf")
                nc.vector.memset(ones_bf, 1.0)
                cT = sgl.tile([P, NH], F32, tag="cT")

                def feat_transpose(x_dram, b, pool, tag, drain):
                    """[L, H] natural f32r -> 8 tiles X^T[h_chunk] of [128, L] fp16.

                    One 512KB DMA per h-chunk; drain on ACT or DVE to balance."""
                    tiles = []
                    for hc in range(NH):
                        t = pool.tile([P, L], F16, tag=tag)
                        bt = blk.tile([P, NH, P], F32R, tag="blk")
                        nc.sync.dma_start(
                            out=bt,
                            in_=x_dram.ap()[
                                b, :, hc * P : (hc + 1) * P
                            ].rearrange("(c p) h -> p c h", p=P),
                        )
                        for g in range(2):
                            pst = pstp.tile([P, 512], F32R, tag="tp")
                            for j in range(4):
                                nc.tensor.transpose(
                                    pst[:, j * P : (j + 1) * P], bt[:, g * 4 + j, :], ident
                                )
                            if drain == "act":
                                nc.scalar.activation(
                                    t[:, g * 512 : (g + 1) * 512], pst, COPY
                                )
                            else:
                                nc.vector.tensor_copy(
                                    t[:, g * 512 : (g + 1) * 512], pst
                                )
                        tiles.append(t)
                    return tiles

                Mt = [mtp.tile([P, H], F16, tag="mt", name=f"mt{i}") for i in range(NH)]
                WvT = [
                    wvtp.tile([P, A], BF16, tag="wvt", name=f"wvt{i}")
                    for i in range(NH)
                ]

                # ---- batch 0 feature transposes first: PE starts on meme
                # ---- blocks while wq/wk stream in behind it.
                memeT = feat_transpose(xm, 0, xmp, "xmt", "dve")

                # ---- one-time: Mt = Wq^T Wk (both natural), c = Wk^T bq,
                # wk streamed in 512-col halves; c folded into the wk pass.
                with (
                    tc.tile_pool(name="wq", bufs=8) as wqp,
                    tc.tile_pool(name="wk", bufs=10) as wkp,
                ):
                    wqn = []
                    for ci in range(NH):
                        tq = wqp.tile([P, H], F32R, tag="wq", name=f"wqn{ci}")
                        nc.sync.dma_start(out=tq, in_=wq.ap()[ci * P : (ci + 1) * P, :])
                        wqn.append(tq)
                    for g in range(2):
                        wkh = []
                        for ci in range(NH):
                            tk = wkp.tile([P, 512], F32R, tag="wk")
                            nc.sync.dma_start(
                                out=tk,
                                in_=wk.ap()[
                                    ci * P : (ci + 1) * P, g * 512 : (g + 1) * 512
                                ],
                            )
                            wkh.append(tk)
                        for h2 in range(NH):
                            pst = psp.tile([P, 512], F32, tag="mm")
                            for ac in range(NH):
                                nc.tensor.matmul(
                                    pst,
                                    lhsT=wqn[ac][:, h2 * P : (h2 + 1) * P],
                                    rhs=wkh[ac],
                                    start=(ac == 0),
                                    stop=(ac == NH - 1),
                                )
                            nc.vector.tensor_copy(
                                Mt[h2][:, g * 512 : (g + 1) * 512], pst
                            )
                        for ht in range(4):
                            psc = ps2.tile([P, 2], F32, tag="sum")
                            for ac in range(NH):
                                nc.tensor.matmul(
                                    psc,
                                    lhsT=wkh[ac][:, ht * P : (ht + 1) * P],
                                    rhs=bqc[:, ac : ac + 2],
                                    start=(ac == 0),
                                    stop=(ac == NH - 1),
                                )
                            nc.vector.tensor_copy(
                                cT[:, g * 4 + ht : g * 4 + ht + 1], psc[:, 0:1]
                            )

                def compute_G(memeT):
                    G = []
                    for ht in range(NH):
                        gt = gp.tile([P, L], F16, tag="g")
                        for qb in range(2):
                            pst = psp.tile([P, 512], F32, tag="mm")
                            for h2 in range(NH):
                                nc.tensor.matmul(
                                    pst,
                                    lhsT=Mt[h2][:, ht * P : (ht + 1) * P],
                                    rhs=memeT[h2][:, qb * 512 : (qb + 1) * 512],
                                    start=(h2 == 0),
                                    stop=(h2 == NH - 1),
                                )
                            nc.vector.tensor_scalar_add(
                                gt[:, qb * 512 : (qb + 1) * 512],
                                pst,
                                cT[:, ht : ht + 1],
                            )
                        G.append(gt)
                    return G

                def load_emoji(b):
                    EM = []
                    for kc in range(NH):
                        emt = emp.tile([P, H], BF16, tag="em")
                        for g in range(2):
                            est = estp.tile([P, 512], F32, tag="estg")
                            nc.gpsimd.dma_start(
                                out=est,
                                in_=xe.ap()[
                                    b, kc * P : (kc + 1) * P, g * 512 : (g + 1) * 512
                                ],
                            )
                            nc.vector.tensor_copy(
                                emt[:, g * 512 : (g + 1) * 512], est
                            )
                        EM.append(emt)
                    return EM

                def attention_qb(b, qb, textT, G, EM):
                    if True:
                        # S^T[k_tile, qb] -> exp -> E^T bf16
                        ets = []
                        for kt in range(NH):
                            pst = psp.tile([P, 512], F32, tag="mm")
                            for hc in range(NH):
                                nc.tensor.matmul(
                                    pst,
                                    lhsT=textT[hc][:, kt * P : (kt + 1) * P],
                                    rhs=G[hc][:, qb * 512 : (qb + 1) * 512],
                                    start=(hc == 0),
                                    stop=(hc == NH - 1),
                                )
                            e_t = etp.tile([P, 512], BF16, tag="et")
                            nc.scalar.activation(e_t, pst, EXP)
                            ets.append(e_t)

                        # T^T[h_tile, qb] = sum_k emoji[k, h] E^T[k, qb]
                        Tt = []
                        for ht in range(NH):
                            pst = psp.tile([P, 512], F32, tag="mm")
                            for kc in range(NH):
                                nc.tensor.matmul(
                                    pst,
                                    lhsT=EM[kc][:, ht * P : (ht + 1) * P],
                                    rhs=ets[kc],
                                    start=(kc == 0),
                                    stop=(kc == NH - 1),
                                )
                            t_t = ttp.tile([P, 512], BF16, tag="tt")
                            nc.vector.tensor_copy(t_t, pst)
                            Tt.append(t_t)

                        # O[q_tile, :] = (sum_h T^T[h,q] WvT[h,a]) / s[q] + bv
                        for qt in range(4):
                            qs = qt * P
                            ps0 = psp.tile([P, 512], F32, tag="mm")
                            ps1 = psp.tile([P, 512], F32, tag="mm")
                            pss = ps2.tile([P, 1], F32, tag="sum")
                            for kc in range(NH):
                                nc.tensor.matmul(
                                    pss,
                                    lhsT=ets[kc][:, qs : qs + P],
                                    rhs=ones_bf,
                                    start=(kc == 0),
                                    stop=(kc == NH - 1),
                                )
                            for hc in range(NH):
                                st, sp = (hc == 0), (hc == NH - 1)
                                nc.tensor.matmul(
                                    ps0,
                                    lhsT=Tt[hc][:, qs : qs + P],
                                    rhs=WvT[hc][:, 0:512],
                                    start=st,
                                    stop=sp,
                                )
                                nc.tensor.matmul(
                                    ps1,
                                    lhsT=Tt[hc][:, qs : qs + P],
                                    rhs=WvT[hc][:, 512:1024],
                                    start=st,
                                    stop=sp,
                                )
                            rec = smp.tile([P, 1], F32, tag="rec")
                            nc.vector.reciprocal(rec, pss)
                            q0 = qb * 512 + qs
                            for g, psg in ((0, ps0), (1, ps1)):
                                o_t = opp.tile([P, 512], F32, tag="op")
                                nc.scalar.activation(o_t, psg, COPY, scale=rec)
                                nc.vector.tensor_add(
                                    o_t, o_t, bvb[:, g * 512 : (g + 1) * 512]
                                )
                                nc.scalar.dma_start(
                                    out=o.ap()[
                                        b, q0 : q0 + P, g * 512 : (g + 1) * 512
                                    ],
                                    in_=o_t,
                                )

                # ---- batch 0 ----
                G0 = compute_G(memeT)
                textT = feat_transpose(xt_, 0, xtp, "xtt", "act")
                # WvT via PE transpose of Wv natural blocks (needed by O stage)
                for hc in range(NH):
                    for g in range(2):
                        pst = pstp.tile([P, 512], F32R, tag="tp")
                        bt = blk.tile([P, 4, P], F32R, tag="blk")
                        nc.sync.dma_start(
                            out=bt,
                            in_=wv.ap()[
                                g * 512 : (g + 1) * 512, hc * P : (hc + 1) * P
                            ].rearrange("(c p) h -> p c h", p=P),
                        )
                        for j in range(4):
                            nc.tensor.transpose(
                                pst[:, j * P : (j + 1) * P], bt[:, j, :], ident
                            )
                        nc.vector.tensor_copy(
                            WvT[hc][:, g * 512 : (g + 1) * 512], pst
                        )
                EM = load_emoji(0)
                attention_qb(0, 0, textT, G0, EM)
                # prefetch batch-1 meme transposes between batch-0 qb passes
                memeT1 = feat_transpose(xm, 1, xmp, "xmt", "dve")
                attention_qb(0, 1, textT, G0, EM)

                # ---- batch 1 ----
                G1 = compute_G(memeT1)
                textT1 = feat_transpose(xt_, 1, xtp, "xtt", "act")
                EM1 = load_emoji(1)
                attention_qb(1, 0, textT1, G1, EM1)
                attention_qb(1, 1, textT1, G1, EM1)

    nc.compile()
    return nc


_NC = {}


def _get_nc(repeat=1):
    if repeat not in _NC:
        _NC[repeat] = _build_program(repeat)
    return _NC[repeat]


def _run(inputs, trace=False, repeat=1):
    nc = _get_nc(repeat)
    c = np.ascontiguousarray

    def f32c(x):
        return c(np.asarray(x, dtype=np.float32))

    meme = f32c(inputs["meme_features"])
    text = f32c(inputs["text_features"])
    emoji = f32c(inputs["emoji_features"])
    full = {
        "wq": f32c(inputs["Wq"]),
        "wk": f32c(inputs["Wk"]),
        "wv": f32c(inputs["Wv"]),
        "bq": f32c(inputs["bq"]),
        "bv": f32c(inputs["bv"]),
    }
    in_maps = []
    for i in range(NCORES):
        s = slice(i * NB, (i + 1) * NB)
        in_maps.append(
            {"xm": c(meme[s]), "xt": c(text[s]), "xe": c(emoji[s]), **full}
        )
    res = run_bass_kernel_spmd(nc, in_maps, list(range(NCORES)), trace=trace)
    out = np.concatenate([res.results[i]["o"] for i in range(NCORES)], axis=0)
    return out, res


def kernel(**inputs):
    out, _ = _run(inputs, trace=False)
    return out


if __name__ == "__main__":
    rng = np.random.default_rng(0)
    s = 1.0 / np.sqrt(H)
    inputs = {
        "meme_features": rng.standard_normal((B, L, H), dtype=np.float32),
        "text_features": rng.standard_normal((B, L, H), dtype=np.float32),
        "emoji_features": rng.standard_normal((B, L, H), dtype=np.float32),
        "Wq": rng.uniform(-s, s, (A, H)).astype(np.float32),
        "bq": rng.uniform(-s, s, A).astype(np.float32),
        "Wk": rng.uniform(-s, s, (A, H)).astype(np.float32),
        "bk": rng.uniform(-s, s, A).astype(np.float32),
        "Wv": rng.uniform(-s, s, (A, H)).astype(np.float32),
        "bv": rng.uniform(-s, s, A).astype(np.float32),
    }
    out = kernel(**inputs)
    q = np.einsum("blh,ah->bla", inputs["meme_features"], inputs["Wq"]) + inputs["bq"]
    k = np.einsum("blh,ah->bla", inputs["text_features"], inputs["Wk"]) + inputs["bk"]
    v = np.einsum("blh,ah->bla", inputs["emoji_features"], inputs["Wv"]) + inputs["bv"]
    sc = np.einsum("bqa,bka->bqk", q, k)
    sc -= sc.max(-1, keepdims=True)
    w = np.exp(sc)
    w /= w.sum(-1, keepdims=True)
    ref = np.einsum("bqk,bka->bqa", w, v)
    err = np.linalg.norm(out - ref) / np.linalg.norm(ref)
    print(f"smoke rel err: {err:.3e}")
